# revision 10
# baseline (speedup 1.0000x reference)
"""GCN classifier (GCNConv + LayerNorm + ReLU + Linear) on 8 Trainium2 NeuronCores.

Strategy (self-contained; sized for N=100000, E=1600000, 128 ch, 16 classes):
  out = LN((A @ x) @ W1 + b1).relu() @ Wfc + bfc,  A = normalized adjacency.
  - Host: add self-loops; per-edge value v = dinv[src]*w quantized to uint8
    with a per-destination-node scale (s = colmax/255); the dst factor
    dinv[dst]*s is folded into the per-tile LayerNorm input scale (a
    per-partition scalar on the stt), so the device only ever sees integer
    one-hot values 0..255 (exact in bf16).  Destination nodes are assigned
    to 784 tiles of 128 slots (LPT-balanced); 98 tiles per core; edges
    sorted by src within each (tile,bank) cell for HBM locality.
  - Device (per core): per group of GS tiles, dma_gather x[src] rows (bf16)
    from 4 source banks; the segment-sum one-hot is either STREAMED from
    DRAM as uint8 with SWDGE cast-DMA to bf16 (queue 0), or BUILT on-device
    with two DVE ops (iota==slot, then *q) — the split is compile-time per
    group to balance DVE vs DMA.  Then per 128-edge chunk one bf16 matmul
    accumulating in PSUM; W1 matmul; LayerNorm (E[x^2]-mu^2 form, scale+
    bias+relu fused into one ACT op; ln_g folded into Wfc on host);
    transpose; Wfc matmul.  One bulk store at the end.
  - Host: concatenate per-core outputs and un-permute node rows.
"""
import os
import heapq
import numpy as np

N_NODES = 100000
IN_CH = 128
HIDDEN = 128
NUM_CLASSES = 16
LN_EPS = 1e-5
N_CORES = 8
P = 128
BANK = 25000
NBANK = 4
GS = 4   # tiles per gather group
FCB = 8  # tiles per fc-output PSUM batch

# --- tuning flags ---
SP_GATHER = False       # single_packet on gathers (True crashes the ucode)
NQ = 4                  # SWDGE queues; gathers use queue = source bank
GATHER_QUEUES = (0, 1, 2, 3)
N_STREAM = 0            # of the 28 groups, how many get a DMA-streamed one-hot
NGRP = 28
REP = 8                 # inner repeat for the 2x-mode one-hot build

LAST_RESULTS = None
_PROGRAM_CACHE = {}


_STREAM_SET = {round(i * NGRP / max(N_STREAM, 1)) for i in range(N_STREAM)}


def _stream_group(g):
    return (g % NGRP) in _STREAM_SET


# ----------------------------------------------------------------------------
# host-side preprocessing
# ----------------------------------------------------------------------------
def _preprocess(edge_index, edge_weight):
    src = np.asarray(edge_index[0], dtype=np.int64)
    dst = np.asarray(edge_index[1], dtype=np.int64)
    w = np.asarray(edge_weight, dtype=np.float32)
    N = N_NODES
    loop = np.arange(N, dtype=np.int64)
    src = np.concatenate([src, loop])
    dst = np.concatenate([dst, loop])
    w = np.concatenate([w, np.ones(N, dtype=np.float32)])

    deg = np.bincount(dst, weights=w.astype(np.float64), minlength=N).astype(np.float32)
    dinv = np.where(deg > 0, 1.0 / np.sqrt(deg), 0.0).astype(np.float32)

    # per-edge value BEFORE dst factor; quantized per dst node to uint8
    v = dinv[src] * w
    colmax = np.zeros(N, dtype=np.float32)
    np.maximum.at(colmax, dst, v)
    colmax = np.maximum(colmax, 1e-30)
    s_node = colmax / 255.0
    q = np.rint(v / s_node[dst]).astype(np.float32)   # 0..255 integers
    unscale = (s_node * dinv).astype(np.float32)      # fold into LN stt scalar

    # --- bank-aware node->tile assignment (greedy LPT on per-bank cell
    # loads, 128-node cap per tile).  896 tiles keeps every (tile,bank)
    # cell under 4*128 edges => a K=(4,4,4,4) chunk profile (fill ~0.93,
    # ~9% fewer gather descriptors / PE matmuls than 784 tiles @ K=5). ---
    TILES = 896
    eb_all = src // BANK
    cntb = np.zeros((N, NBANK), dtype=np.int64)
    np.add.at(cntb, (dst, eb_all), 1)
    cnt = cntb.sum(1)
    order = np.argsort(-cnt, kind="stable")
    loads = np.zeros((TILES, NBANK), dtype=np.int64)
    node_cnt = np.zeros(TILES, dtype=np.int64)
    node_tile = np.empty(N, dtype=np.int64)
    node_slot = np.empty(N, dtype=np.int64)
    heap = [(0, 0, t) for t in range(TILES)]
    heapq.heapify(heap)
    CAND = 6
    for nd in order:
        c = cntb[nd]
        cands = []
        while len(cands) < CAND and heap:
            mb, tot, t = heapq.heappop(heap)
            if node_cnt[t] >= P:
                continue
            cur = loads[t]
            curmb = cur.max()
            if curmb != mb:
                heapq.heappush(heap, (int(curmb), int(cur.sum()), t))
                continue
            cands.append(t)
        best, bkey = None, None
        for t in cands:
            nl = loads[t] + c
            key = (nl.max(), nl.sum())
            if bkey is None or key < bkey:
                bkey, best = key, t
        for t in cands:
            if t != best:
                heapq.heappush(
                    heap, (int(loads[t].max()), int(loads[t].sum()), t))
        node_tile[nd] = best
        node_slot[nd] = node_cnt[best]
        loads[best] += c
        node_cnt[best] += 1
        if node_cnt[best] < P:
            heapq.heappush(
                heap, (int(loads[best].max()), int(loads[best].sum()), best))

    TPC = TILES // N_CORES

    # --- per-(tile,bank) groups and static chunk profile ---
    et = node_tile[dst]                      # tile of each edge
    eb = src // BANK                         # source bank of each edge
    cell = np.zeros((TILES, NBANK), dtype=np.int64)
    np.add.at(cell, (et, eb), 1)
    K = (-(-cell // P)).max(axis=0)          # static chunks per bank
    Koff = np.zeros(NBANK + 1, dtype=np.int64)
    np.cumsum(K, out=Koff[1:])
    CH = int(Koff[-1])                       # chunks per tile

    # position of each edge within its (tile, bank) cell, src-sorted within
    # the cell so gather descriptors walk HBM mostly in ascending order
    keys = et * NBANK + eb
    eorder = np.lexsort((src, keys))
    keys_s = keys[eorder]
    gs2 = np.zeros(TILES * NBANK + 1, dtype=np.int64)
    np.cumsum(np.bincount(keys_s, minlength=TILES * NBANK), out=gs2[1:])
    pos = np.arange(len(keys_s)) - gs2[keys_s]

    src_s = src[eorder]
    dst_s = dst[eorder]
    et_s = et[eorder]
    eb_s = eb[eorder]
    q_s = q[eorder]

    kk = pos // P          # chunk within (tile,bank)
    lane = pos % P
    assert (kk < K[eb_s]).all(), "bank profile overflow; increase capacity"

    tl = et_s % TPC        # core-local tile
    g = tl // GS
    j = tl % GS
    Sg = np.minimum(GS, TPC - g * GS)
    col_local = g * GS * CH + Sg * Koff[eb_s] + j * K[eb_s] + kk
    col = (et_s // TPC) * TPC * CH + col_local  # gather-order chunk col (idx)
    mcol = et_s * CH + Koff[eb_s] + kk          # tile-major chunk col (meta)

    TOTC = TILES * CH

    idx16 = np.zeros((16, TOTC * 8), dtype=np.int16)
    idx16[lane % 16, col * 8 + lane // 16] = (src_s % BANK).astype(np.int16)
    idx_all = np.tile(idx16, (8, 1))

    # per-tile unscale vector [P, TILES]: slot-major within tile
    scl = np.zeros((P, TILES), dtype=np.float32)
    scl[node_slot, node_tile] = unscale

    return dict(
        idx_all=idx_all,
        oh_lane=lane, oh_mcol=mcol, oh_slot=node_slot[dst_s], oh_val=q_s,
        node_tile=node_tile, node_slot=node_slot, scl=scl,
        TILES=TILES, CHUNKS=CH, TPC=TPC,
        K=tuple(int(k) for k in K),
    )


def _groups(TPC):
    out = []
    t = 0
    g = 0
    while t < TPC:
        s = min(GS, TPC - t)
        out.append((s, g * GS))
        t += s
        g += 1
    return out


# ----------------------------------------------------------------------------
# device program
# ----------------------------------------------------------------------------
def _build_program(TPC, CH, K, beta_zero, bfc_zero, sgn_needed):
    from contextlib import ExitStack
    import concourse.bass as bass
    import concourse.tile as tile
    from concourse import bacc, mybir

    f32 = mybir.dt.float32
    bf16 = mybir.dt.bfloat16
    i16 = mybir.dt.int16
    u8 = mybir.dt.uint8
    NCOLS = TPC * CH
    Koff = [0]
    for k in K:
        Koff.append(Koff[-1] + k)
    grps = _groups(TPC)
    n_stream = sum(1 for gi in range(len(grps)) if _stream_group(gi))
    any_stream = n_stream > 0
    any_dve = n_stream < len(grps)
    # DRAM column base of each streamed group's one-hot block
    stream_base = {}
    acc = 0
    for gi, (s, gb) in enumerate(grps):
        if _stream_group(gi):
            stream_base[gi] = acc
            acc += s * CH
    SCOLS = max(acc, 1)

    nc = bacc.Bacc("TRN2", target_bir_lowering=False, debug=False,
                   num_devices=N_CORES, num_swdge_queues=NQ)
    xb = [nc.dram_tensor(f"xb{b}", [BANK, IN_CH], bf16, kind="ExternalInput").ap()
          for b in range(NBANK)]
    idx_d = nc.dram_tensor("idx", [P, NCOLS * 8], i16, kind="ExternalInput").ap()
    slot_d = nc.dram_tensor("slotm", [P, NCOLS], bf16, kind="ExternalInput").ap()
    nrm_d = nc.dram_tensor("nrmm", [P, NCOLS], bf16, kind="ExternalInput").ap()
    ohq_d = nc.dram_tensor("ohq", [P, SCOLS * P], u8, kind="ExternalInput").ap()
    scl_d = nc.dram_tensor("scl", [P, TPC], f32, kind="ExternalInput").ap()
    w1_d = nc.dram_tensor("W1", [IN_CH, HIDDEN], bf16, kind="ExternalInput").ap()
    wfc_d = nc.dram_tensor("Wfc", [HIDDEN, NUM_CLASSES], bf16, kind="ExternalInput").ap()
    b1_db = nc.dram_tensor("b1", [1, HIDDEN], bf16, kind="ExternalInput").ap()
    beta_d = nc.dram_tensor("beta", [1, HIDDEN], f32, kind="ExternalInput").ap()
    sgn_d = nc.dram_tensor("sgn", [1, HIDDEN], f32, kind="ExternalInput").ap()
    bfc_d = nc.dram_tensor("bfc", [1, NUM_CLASSES], f32, kind="ExternalInput").ap()
    idm_d = nc.dram_tensor("idm", [P, P], bf16, kind="ExternalInput").ap()
    iot_d = nc.dram_tensor("iot", [P, P], bf16, kind="ExternalInput").ap()
    out_d = nc.dram_tensor("out", [TPC * P, NUM_CLASSES], f32,
                           kind="ExternalOutput").ap()

    def bcast(src_ap, parts=P):
        return bass.AP(tensor=src_ap.tensor, offset=src_ap.offset,
                       ap=[[0, parts]] + list(src_ap.ap[1:]))

    AL = mybir.AluOpType
    AF = mybir.ActivationFunctionType

    with tile.TileContext(nc) as tc, ExitStack() as ctx:
        consts = ctx.enter_context(tc.tile_pool(name="consts", bufs=1))
        gpool = ctx.enter_context(tc.tile_pool(name="gather", bufs=3))
        ohpool = ctx.enter_context(tc.tile_pool(name="onehot", bufs=2))
        ohupool = ctx.enter_context(tc.tile_pool(name="ohu", bufs=1))
        sp = ctx.enter_context(tc.tile_pool(name="work", bufs=4))
        t1pool = ctx.enter_context(tc.tile_pool(name="t1p", bufs=FCB + 3))
        statp = ctx.enter_context(tc.tile_pool(name="stats", bufs=3))
        pp_ps = ctx.enter_context(tc.tile_pool(name="pp_ps", bufs=2, space="PSUM"))
        agg_ps = ctx.enter_context(tc.tile_pool(name="agg_ps", bufs=2, space="PSUM"))
        tr_ps = ctx.enter_context(tc.tile_pool(name="tr_ps", bufs=2, space="PSUM"))
        fc_ps = ctx.enter_context(tc.tile_pool(name="fc_ps", bufs=2, space="PSUM"))

        W1_s = consts.tile([IN_CH, HIDDEN], bf16)
        nc.sync.dma_start(W1_s[:], w1_d[:])
        Wfc_s = consts.tile([HIDDEN, NUM_CLASSES], bf16)
        nc.sync.dma_start(Wfc_s[:], wfc_d[:])
        B1 = consts.tile([P, HIDDEN], bf16)
        nc.sync.dma_start(B1[:], bcast(b1_db))
        if not beta_zero:
            BETA = consts.tile([P, HIDDEN], f32)
            nc.sync.dma_start(BETA[:], bcast(beta_d))
        if sgn_needed:
            SGN = consts.tile([P, HIDDEN], f32)
            nc.sync.dma_start(SGN[:], bcast(sgn_d))
        if not bfc_zero:
            BFC = consts.tile([P, FCB * NUM_CLASSES], f32)
            nc.sync.dma_start(
                BFC[:], bass.AP(tensor=bfc_d.tensor, offset=bfc_d.offset,
                                ap=[[0, P], [0, FCB], [1, NUM_CLASSES]]))
        ident = consts.tile([P, P], bf16)
        nc.sync.dma_start(ident[:], idm_d[:])
        eps_t = consts.tile([P, FCB], f32)
        nc.vector.memset(eps_t[:], LN_EPS)
        scl_s = consts.tile([P, TPC], f32)
        nc.sync.dma_start(scl_s[:], scl_d[:])

        idx_s = consts.tile([P, NCOLS * 8], i16)
        nc.sync.dma_start(idx_s[:], idx_d[:])
        built_base = {}
        bacc_cols = 0
        for gi2, (s2, gb2) in enumerate(grps):
            if not _stream_group(gi2):
                built_base[gi2] = bacc_cols
                bacc_cols += s2 * CH
        BCOLS = max(bacc_cols, 1)
        if any_dve:
            IOT = consts.tile([P, P], bf16)
            nc.sync.dma_start(IOT[:], iot_d[:])
            slot_s = consts.tile([P, NCOLS], bf16)
            nc.sync.dma_start(slot_s[:], slot_d[:])
            nrm_s = consts.tile([P, NCOLS], bf16)
            nc.sync.dma_start(nrm_s[:], nrm_d[:])
            srep = consts.tile([P, BCOLS * REP], bf16)
            nrep = consts.tile([P, BCOLS * REP], bf16)

        out_acc = consts.tile([P, TPC * NUM_CLASSES], f32)

        ohq_v = ohq_d.rearrange("p (t c) -> p t c", c=P)

        t_global = 0
        qn = 0
        fc = None
        nb = 0
        for gi, (s, gbase_tile) in enumerate(grps):
            gbase = gbase_tile * CH       # chunk-column base of this group
            nco = s * CH
            Gg = gpool.tile([P, GS * CH, IN_CH], bf16, tag="Gg")
            gq = GATHER_QUEUES if any_stream else tuple(range(NQ))
            for b in range(NBANK):
                n = s * K[b] * P
                ccol = gbase + s * Koff[b]
                nc.gpsimd.dma_gather(
                    out_ap=Gg[:, s * Koff[b]:s * Koff[b] + s * K[b], :],
                    in_ap=xb[b][:],
                    idxs_ap=idx_s[:, ccol * 8:ccol * 8 + n // 16],
                    num_idxs=n, num_idxs_reg=n, elem_size=IN_CH,
                    single_packet=SP_GATHER, queue_num=gq[qn % len(gq)],
                )
                qn += 1
            ohg = ohpool.tile([P, GS * CH, P], bf16, tag="ohg")
            if _stream_group(gi):
                # uint8 one-hot streamed via HWDGE, cast u8->bf16 on DVE
                sb = stream_base[gi]
                ohu = ohupool.tile([P, GS * CH, P], u8, tag="ohu")
                nc.sync.dma_start(ohu[:, :nco, :], ohq_v[:, sb:sb + nco, :])
                nc.vector.tensor_copy(ohg[:, :nco, :], ohu[:, :nco, :])
            else:
                # built on device: oh[p,c,j] = (iota[j]==slot[p,c]) * q[p,c];
                # slot/q are pre-expanded x8 so every operand has a unit
                # innermost AP step (DVE 2x perf mode).
                bb = built_base[gi]
                sv = srep[:, bb * REP:(bb + nco) * REP]
                nv = nrep[:, bb * REP:(bb + nco) * REP]
                nc.vector.tensor_copy(
                    sv.rearrange("p (c r) -> p c r", r=REP),
                    slot_s[:, gbase:gbase + nco].unsqueeze(2).broadcast_to(
                        [P, nco, REP]))
                nc.vector.tensor_copy(
                    nv.rearrange("p (c r) -> p c r", r=REP),
                    nrm_s[:, gbase:gbase + nco].unsqueeze(2).broadcast_to(
                        [P, nco, REP]))
                ov = ohg[:, :nco, :].rearrange("p c (jb j8) -> p c jb j8",
                                               j8=REP)
                iota_v = bass.AP(tensor=IOT.tensor, offset=IOT[:].offset,
                                 ap=[IOT[:].ap[0], [0, nco],
                                     [REP, P // REP], [1, REP]])
                slot_v = bass.AP(tensor=srep.tensor, offset=sv.offset,
                                 ap=[sv.ap[0], [REP, nco],
                                     [0, P // REP], [1, REP]])
                nrm_v = bass.AP(tensor=nrep.tensor, offset=nv.offset,
                                ap=[nv.ap[0], [REP, nco],
                                    [0, P // REP], [1, REP]])
                nc.vector.tensor_tensor(out=ov, in0=iota_v, in1=slot_v,
                                        op=AL.is_equal)
                nc.vector.tensor_tensor(out=ov, in0=ov, in1=nrm_v,
                                        op=AL.mult)
            for j in range(s):
                t = t_global
                t_global += 1
                Pp = pp_ps.tile([IN_CH, P], f32, space="PSUM")
                mm = 0
                for b in range(NBANK):
                    for kk in range(K[b]):
                        cig = s * Koff[b] + j * K[b] + kk
                        nc.tensor.matmul(Pp[:], lhsT=Gg[:, cig, :],
                                         rhs=ohg[:, j * CH + mm, :],
                                         start=(mm == 0), stop=(mm == CH - 1))
                        mm += 1
                Ps = sp.tile([IN_CH, P], bf16, tag="Ps")
                nc.scalar.activation(out=Ps[:], in_=Pp[:], func=AF.Copy)
                agg = agg_ps.tile([P, HIDDEN], f32, space="PSUM")
                nc.tensor.matmul(agg[:], lhsT=Ps[:], rhs=W1_s[:],
                                 start=True, stop=True)
                jb = t % FCB
                if jb == 0:
                    nb = min(FCB, TPC - t)
                    fc = fc_ps.tile([P, FCB * NUM_CLASSES], f32, space="PSUM")
                    musum_b = statp.tile([P, FCB], f32, tag="musum")
                    s2sum_b = statp.tile([P, FCB], f32, tag="s2sum")
                    rstd_b = statp.tile([P, FCB], f32, tag="rstd")
                    nmr_b = statp.tile([P, FCB], f32, tag="nmr")
                    t1s = []
                # LayerNorm with var = E[x^2] - mu^2; quantization unscale
                # folded into the stt per-partition scalar; per-tile stats
                # batched FCB tiles at a time to amortize tiny-op overhead.
                t1 = t1pool.tile([P, HIDDEN], f32, tag="t1")
                t1s.append(t1)
                nc.vector.scalar_tensor_tensor(
                    out=t1[:], in0=agg[:], scalar=scl_s[:, t:t + 1],
                    in1=B1[:], op0=AL.mult, op1=AL.add,
                    accum_out=musum_b[:, jb:jb + 1])
                sq = sp.tile([P, HIDDEN], f32, tag="sq")
                nc.scalar.activation(out=sq[:], in_=t1[:], func=AF.Square,
                                     accum_out=s2sum_b[:, jb:jb + 1])
                if jb == nb - 1:
                    mm2 = statp.tile([P, FCB], f32, tag="mm2")
                    nc.vector.tensor_tensor(
                        out=mm2[:, :nb], in0=musum_b[:, :nb],
                        in1=musum_b[:, :nb], op=AL.mult)
                    varv = statp.tile([P, FCB], f32, tag="varv")
                    nc.vector.scalar_tensor_tensor(
                        out=varv[:, :nb], in0=mm2[:, :nb],
                        scalar=-1.0 / (HIDDEN * HIDDEN),
                        in1=eps_t[:, :nb], op0=AL.mult, op1=AL.add)
                    nc.vector.scalar_tensor_tensor(
                        out=varv[:, :nb], in0=s2sum_b[:, :nb],
                        scalar=1.0 / HIDDEN,
                        in1=varv[:, :nb], op0=AL.mult, op1=AL.add)
                    nc.scalar.activation(out=rstd_b[:, :nb], in_=varv[:, :nb],
                                         func=AF.Sqrt)
                    nc.vector.reciprocal(out=rstd_b[:, :nb],
                                         in_=rstd_b[:, :nb])
                    nc.vector.scalar_tensor_tensor(
                        out=nmr_b[:, :nb], in0=musum_b[:, :nb],
                        scalar=-1.0 / HIDDEN,
                        in1=rstd_b[:, :nb], op0=AL.mult, op1=AL.mult)
                    for j2 in range(nb):
                        t2 = t - jb + j2
                        t1j = t1s[j2]
                        hr = sp.tile([P, HIDDEN], bf16, tag="hr")
                        if beta_zero and not sgn_needed:
                            nc.scalar.activation(
                                out=hr[:], in_=t1j[:], func=AF.Relu,
                                bias=nmr_b[:, j2:j2 + 1],
                                scale=rstd_b[:, j2:j2 + 1])
                        else:
                            z = sp.tile([P, HIDDEN], f32, tag="z")
                            nc.scalar.activation(
                                out=z[:], in_=t1j[:], func=AF.Identity,
                                bias=nmr_b[:, j2:j2 + 1],
                                scale=rstd_b[:, j2:j2 + 1])
                            if sgn_needed:
                                nc.vector.tensor_tensor(
                                    out=z[:], in0=z[:], in1=SGN[:], op=AL.mult)
                            if not beta_zero:
                                nc.vector.tensor_tensor(
                                    out=z[:], in0=z[:], in1=BETA[:], op=AL.add)
                            nc.scalar.activation(out=hr[:], in_=z[:],
                                                 func=AF.Relu)
                        hrT_ps = tr_ps.tile([HIDDEN, P], bf16, space="PSUM")
                        nc.tensor.transpose(out=hrT_ps[:], in_=hr[:],
                                            identity=ident[:])
                        hrT = sp.tile([HIDDEN, P], bf16, tag="hrT")
                        nc.scalar.activation(out=hrT[:], in_=hrT_ps[:],
                                             func=AF.Copy)
                        nc.tensor.matmul(
                            fc[:, j2 * NUM_CLASSES:(j2 + 1) * NUM_CLASSES],
                            lhsT=hrT[:], rhs=Wfc_s[:], start=True, stop=True)
                    t0 = t - jb
                    dstv = out_acc[:, t0 * NUM_CLASSES:(t + 1) * NUM_CLASSES]
                    srcv = fc[:, :nb * NUM_CLASSES]
                    if bfc_zero:
                        nc.vector.tensor_copy(dstv, srcv)
                    else:
                        nc.vector.tensor_tensor(
                            out=dstv, in0=srcv,
                            in1=BFC[:, :nb * NUM_CLASSES], op=AL.add)

        out_view = out_d.rearrange("(t p) c -> p t c", p=P)
        acc_view = out_acc[:].rearrange("p (t c) -> p t c", c=NUM_CLASSES)
        nc.sync.dma_start(out_view, acc_view)

    nc.compile()
    return nc


def _ensure_ntff_hook():
    import sys, types
    try:
        from antenv.axon_hooks import get_axon_ntff_profile_hook  # noqa: F401
        return
    except ImportError:
        pass
    mod = types.ModuleType("antenv.axon_hooks")
    _hook = [None]
    mod.set_axon_ntff_profile_hook = lambda h: _hook.__setitem__(0, h)
    mod.get_axon_ntff_profile_hook = lambda: _hook[0]
    sys.modules["antenv.axon_hooks"] = mod
    try:
        import antenv
        antenv.axon_hooks = mod
    except ImportError:
        pass
    try:
        from trn_agent_boot.trn_boot import _ntff_profile_via_ctypes
        mod.set_axon_ntff_profile_hook(
            _ntff_profile_via_ctypes("/opt/axon/libaxon_pjrt.so"))
    except Exception:
        pass


# ----------------------------------------------------------------------------
# entry point
# ----------------------------------------------------------------------------
def kernel(x, edge_index, edge_weight, W1, b1, ln_g, ln_b, Wfc, bfc):
    global LAST_RESULTS
    import ml_dtypes
    from concourse.bass_utils import run_bass_kernel_spmd

    bf = ml_dtypes.bfloat16
    x = np.asarray(x, dtype=np.float32)
    meta = _preprocess(edge_index, edge_weight)
    TPC, CH, K = meta["TPC"], meta["CHUNKS"], meta["K"]

    # fold ln_g into Wfc: relu(z*g + b) @ Wfc = relu(sgn*z + b/|g|) @ (|g| Wfc)
    ln_g = np.asarray(ln_g, np.float32)
    ln_b = np.asarray(ln_b, np.float32)
    Wfc = np.asarray(Wfc, np.float32)
    bfc = np.asarray(bfc, np.float32)
    gz = ln_g == 0.0
    absg = np.where(gz, 1.0, np.abs(ln_g))
    beta = np.where(gz, 0.0, ln_b / absg)
    sgn = np.where(gz, 0.0, np.sign(ln_g)).astype(np.float32)
    Wfc_f = Wfc * np.where(gz, 0.0, np.abs(ln_g))[:, None]
    bfc_f = bfc + np.maximum(ln_b, 0.0)[gz] @ Wfc[gz] if gz.any() else bfc
    beta_zero = bool((beta == 0.0).all())
    sgn_needed = bool((sgn != 1.0).any())
    bfc_zero = bool((bfc_f == 0.0).all())

    key = (TPC, CH, K, beta_zero, bfc_zero, sgn_needed,
           SP_GATHER, N_STREAM)
    if key not in _PROGRAM_CACHE:
        _PROGRAM_CACHE[key] = _build_program(TPC, CH, K, beta_zero, bfc_zero,
                                             sgn_needed)
    nc = _PROGRAM_CACHE[key]

    NCOLS = TPC * CH
    grps = _groups(TPC)
    banks = {}
    for b in range(NBANK):
        blk = np.zeros((BANK, IN_CH), dtype=bf)
        seg = x[b * BANK:(b + 1) * BANK]
        blk[:len(seg)] = seg.astype(bf)
        banks[f"xb{b}"] = blk
    common = dict(
        banks,
        W1=np.ascontiguousarray(np.asarray(W1, np.float32).astype(bf)),
        Wfc=np.ascontiguousarray(Wfc_f.astype(bf)),
        b1=np.asarray(b1, np.float32).reshape(1, HIDDEN).astype(bf),
        beta=beta.reshape(1, HIDDEN).astype(np.float32),
        sgn=sgn.reshape(1, HIDDEN),
        bfc=bfc_f.reshape(1, NUM_CLASSES).astype(np.float32),
        idm=np.eye(P, dtype=np.float32).astype(bf),
        iot=np.tile(np.arange(P, dtype=np.float32).astype(bf), (P, 1)),
    )
    # per-core one-hot metadata (quantized values q, exact in bf16)
    oh_lane = meta["oh_lane"]
    oh_mcol = meta["oh_mcol"]
    oh_slot = meta["oh_slot"]
    oh_val = meta["oh_val"]
    core_of = oh_mcol // NCOLS

    # map tile-major metadata col -> streamed DRAM col (or -1)
    col2stream = np.full(NCOLS, -1, dtype=np.int64)
    acc = 0
    for gi, (s, gb) in enumerate(grps):
        if _stream_group(gi):
            col2stream[gb * CH:(gb + s) * CH] = np.arange(acc, acc + s * CH)
            acc += s * CH
    SCOLS = max(acc, 1)

    in_maps = []
    for core in range(N_CORES):
        sl8 = slice(core * NCOLS * 8, (core + 1) * NCOLS * 8)
        m = core_of == core
        lc = oh_mcol[m] - core * NCOLS
        slotm = np.full((P, NCOLS), -1.0, dtype=bf)
        nrmm = np.zeros((P, NCOLS), dtype=bf)
        slotm[oh_lane[m], lc] = oh_slot[m].astype(bf)
        nrmm[oh_lane[m], lc] = oh_val[m].astype(bf)
        ohq = np.zeros((P, SCOLS * P), dtype=np.uint8)
        sc = col2stream[lc]
        ms = sc >= 0
        ohq[oh_lane[m][ms], sc[ms] * P + oh_slot[m][ms]] = \
            oh_val[m][ms].astype(np.uint8)
        scl = np.ascontiguousarray(
            meta["scl"][:, core * TPC:(core + 1) * TPC])
        in_maps.append(dict(
            common,
            idx=np.ascontiguousarray(meta["idx_all"][:, sl8]),
            slotm=slotm,
            nrmm=nrmm,
            ohq=ohq,
            scl=scl,
        ))

    trace = bool(os.environ.get("KERNEL_TRACE"))
    if trace:
        _ensure_ntff_hook()
    res = run_bass_kernel_spmd(nc, in_maps, list(range(N_CORES)), trace=trace)
    LAST_RESULTS = res

    all_rows = np.concatenate([res.results[c]["out"] for c in range(N_CORES)],
                              axis=0)
    rows = meta["node_tile"] * P + meta["node_slot"]
    return np.ascontiguousarray(all_rows[rows])


# revision 12
# speedup vs baseline: 1.1486x; 1.1486x over previous
"""GCN classifier (GCNConv + LayerNorm + ReLU + Linear) on 8 Trainium2 NeuronCores.

Strategy (self-contained; sized for N=100000, E=1600000, 128 ch, 16 classes):
  out = LN((A @ x) @ W1 + b1).relu() @ Wfc + bfc,  A = normalized adjacency.
  - Host: add self-loops; per-edge value v = dinv[src]*w quantized to uint8
    with a per-destination-node scale (s = colmax/255); the dst factor
    dinv[dst]*s is folded into the per-tile LayerNorm input scale (a
    per-partition scalar on the stt), so the device only ever sees integer
    one-hot values 0..255 (exact in bf16).  Destination nodes are assigned
    to 784 tiles of 128 slots (LPT-balanced); 98 tiles per core; edges
    sorted by src within each (tile,bank) cell for HBM locality.
  - Device (per core): per group of GS tiles, dma_gather x[src] rows (bf16)
    from 4 source banks; the segment-sum one-hot is either STREAMED from
    DRAM as uint8 with SWDGE cast-DMA to bf16 (queue 0), or BUILT on-device
    with two DVE ops (iota==slot, then *q) — the split is compile-time per
    group to balance DVE vs DMA.  Then per 128-edge chunk one bf16 matmul
    accumulating in PSUM; W1 matmul; LayerNorm (E[x^2]-mu^2 form, scale+
    bias+relu fused into one ACT op; ln_g folded into Wfc on host);
    transpose; Wfc matmul.  One bulk store at the end.
  - Host: concatenate per-core outputs and un-permute node rows.
"""
import os
import heapq
import numpy as np

N_NODES = 100000
IN_CH = 128
HIDDEN = 128
NUM_CLASSES = 16
LN_EPS = 1e-5
N_CORES = 8
P = 128
BANK = 25000
NBANK = 4
GS = 4   # tiles per gather group
FCB = 8  # tiles per fc-output PSUM batch

# --- tuning flags ---
SP_GATHER = False       # single_packet on gathers (True crashes the ucode)
NQ = 4                  # SWDGE queues; gathers use queue = source bank
GATHER_QUEUES = (0, 1, 2, 3)
N_STREAM = 21           # of the 36 groups, how many get a DMA-streamed one-hot
NGRP = 36
REP = 8                 # inner repeat for the 2x-mode one-hot build

LAST_RESULTS = None
_PROGRAM_CACHE = {}


_STREAM_SET = {round(i * NGRP / max(N_STREAM, 1)) for i in range(N_STREAM)}


def _stream_group(g):
    return (g % NGRP) in _STREAM_SET


# ----------------------------------------------------------------------------
# host-side preprocessing
# ----------------------------------------------------------------------------
def _preprocess(edge_index, edge_weight):
    src = np.asarray(edge_index[0], dtype=np.int64)
    dst = np.asarray(edge_index[1], dtype=np.int64)
    w = np.asarray(edge_weight, dtype=np.float32)
    N = N_NODES
    loop = np.arange(N, dtype=np.int64)
    src = np.concatenate([src, loop])
    dst = np.concatenate([dst, loop])
    w = np.concatenate([w, np.ones(N, dtype=np.float32)])

    deg = np.bincount(dst, weights=w.astype(np.float64), minlength=N).astype(np.float32)
    dinv = np.where(deg > 0, 1.0 / np.sqrt(deg), 0.0).astype(np.float32)

    # per-edge value BEFORE dst factor; quantized per dst node to uint8
    v = dinv[src] * w
    colmax = np.zeros(N, dtype=np.float32)
    np.maximum.at(colmax, dst, v)
    colmax = np.maximum(colmax, 1e-30)
    s_node = colmax / 255.0
    q = np.rint(v / s_node[dst]).astype(np.float32)   # 0..255 integers
    unscale = (s_node * dinv).astype(np.float32)      # fold into LN stt scalar

    # --- bank-aware node->tile assignment (greedy LPT on per-bank cell
    # loads, 128-node cap per tile).  1152 tiles keeps every (tile,bank)
    # cell under 3*128 edges => a K=(3,3,3,3) chunk profile (fill ~0.96,
    # fewest gather descriptors / PE matmuls). ---
    TILES = 1152
    eb_all = src // BANK
    cntb = np.zeros((N, NBANK), dtype=np.int64)
    np.add.at(cntb, (dst, eb_all), 1)
    cnt = cntb.sum(1)
    order = np.argsort(-cnt, kind="stable")
    loads = np.zeros((TILES, NBANK), dtype=np.int64)
    node_cnt = np.zeros(TILES, dtype=np.int64)
    node_tile = np.empty(N, dtype=np.int64)
    node_slot = np.empty(N, dtype=np.int64)
    heap = [(0, 0, t) for t in range(TILES)]
    heapq.heapify(heap)
    CAND = 6
    for nd in order:
        c = cntb[nd]
        cands = []
        while len(cands) < CAND and heap:
            mb, tot, t = heapq.heappop(heap)
            if node_cnt[t] >= P:
                continue
            cur = loads[t]
            curmb = cur.max()
            if curmb != mb:
                heapq.heappush(heap, (int(curmb), int(cur.sum()), t))
                continue
            cands.append(t)
        best, bkey = None, None
        for t in cands:
            nl = loads[t] + c
            key = (nl.max(), nl.sum())
            if bkey is None or key < bkey:
                bkey, best = key, t
        for t in cands:
            if t != best:
                heapq.heappush(
                    heap, (int(loads[t].max()), int(loads[t].sum()), t))
        node_tile[nd] = best
        node_slot[nd] = node_cnt[best]
        loads[best] += c
        node_cnt[best] += 1
        if node_cnt[best] < P:
            heapq.heappush(
                heap, (int(loads[best].max()), int(loads[best].sum()), best))

    TPC = TILES // N_CORES

    # --- per-(tile,bank) groups and static chunk profile ---
    et = node_tile[dst]                      # tile of each edge
    eb = src // BANK                         # source bank of each edge
    cell = np.zeros((TILES, NBANK), dtype=np.int64)
    np.add.at(cell, (et, eb), 1)
    K = (-(-cell // P)).max(axis=0)          # static chunks per bank
    Koff = np.zeros(NBANK + 1, dtype=np.int64)
    np.cumsum(K, out=Koff[1:])
    CH = int(Koff[-1])                       # chunks per tile

    # position of each edge within its (tile, bank) cell, src-sorted within
    # the cell so gather descriptors walk HBM mostly in ascending order
    keys = et * NBANK + eb
    eorder = np.lexsort((src, keys))
    keys_s = keys[eorder]
    gs2 = np.zeros(TILES * NBANK + 1, dtype=np.int64)
    np.cumsum(np.bincount(keys_s, minlength=TILES * NBANK), out=gs2[1:])
    pos = np.arange(len(keys_s)) - gs2[keys_s]

    src_s = src[eorder]
    dst_s = dst[eorder]
    et_s = et[eorder]
    eb_s = eb[eorder]
    q_s = q[eorder]

    kk = pos // P          # chunk within (tile,bank)
    lane = pos % P
    assert (kk < K[eb_s]).all(), "bank profile overflow; increase capacity"

    tl = et_s % TPC        # core-local tile
    g = tl // GS
    j = tl % GS
    Sg = np.minimum(GS, TPC - g * GS)
    col_local = g * GS * CH + Sg * Koff[eb_s] + j * K[eb_s] + kk
    col = (et_s // TPC) * TPC * CH + col_local  # gather-order chunk col (idx)
    mcol = et_s * CH + Koff[eb_s] + kk          # tile-major chunk col (meta)

    TOTC = TILES * CH

    idx16 = np.zeros((16, TOTC * 8), dtype=np.int16)
    idx16[lane % 16, col * 8 + lane // 16] = (src_s % BANK).astype(np.int16)
    idx_all = np.tile(idx16, (8, 1))

    # per-tile unscale vector [P, TILES]: slot-major within tile
    scl = np.zeros((P, TILES), dtype=np.float32)
    scl[node_slot, node_tile] = unscale

    return dict(
        idx_all=idx_all,
        oh_lane=lane, oh_mcol=mcol, oh_slot=node_slot[dst_s], oh_val=q_s,
        node_tile=node_tile, node_slot=node_slot, scl=scl,
        TILES=TILES, CHUNKS=CH, TPC=TPC,
        K=tuple(int(k) for k in K),
    )


def _groups(TPC):
    out = []
    t = 0
    g = 0
    while t < TPC:
        s = min(GS, TPC - t)
        out.append((s, g * GS))
        t += s
        g += 1
    return out


# ----------------------------------------------------------------------------
# device program
# ----------------------------------------------------------------------------
def _build_program(TPC, CH, K, beta_zero, bfc_zero, sgn_needed):
    from contextlib import ExitStack
    import concourse.bass as bass
    import concourse.tile as tile
    from concourse import bacc, mybir

    f32 = mybir.dt.float32
    bf16 = mybir.dt.bfloat16
    i16 = mybir.dt.int16
    u8 = mybir.dt.uint8
    NCOLS = TPC * CH
    Koff = [0]
    for k in K:
        Koff.append(Koff[-1] + k)
    grps = _groups(TPC)
    n_stream = sum(1 for gi in range(len(grps)) if _stream_group(gi))
    any_stream = n_stream > 0
    any_dve = n_stream < len(grps)
    # DRAM column base of each streamed group's one-hot block
    stream_base = {}
    acc = 0
    for gi, (s, gb) in enumerate(grps):
        if _stream_group(gi):
            stream_base[gi] = acc
            acc += s * CH
    SCOLS = max(acc, 1)

    nc = bacc.Bacc("TRN2", target_bir_lowering=False, debug=False,
                   num_devices=N_CORES, num_swdge_queues=NQ)
    xb = [nc.dram_tensor(f"xb{b}", [BANK, IN_CH], bf16, kind="ExternalInput").ap()
          for b in range(NBANK)]
    idx_d = nc.dram_tensor("idx", [P, NCOLS * 8], i16, kind="ExternalInput").ap()
    slot_d = nc.dram_tensor("slotm", [P, NCOLS], bf16, kind="ExternalInput").ap()
    nrm_d = nc.dram_tensor("nrmm", [P, NCOLS], bf16, kind="ExternalInput").ap()
    ohq_d = nc.dram_tensor("ohq", [P, SCOLS * P], u8, kind="ExternalInput").ap()
    scl_d = nc.dram_tensor("scl", [P, TPC], f32, kind="ExternalInput").ap()
    w1_d = nc.dram_tensor("W1", [IN_CH, HIDDEN], bf16, kind="ExternalInput").ap()
    wfc_d = nc.dram_tensor("Wfc", [HIDDEN, NUM_CLASSES], bf16, kind="ExternalInput").ap()
    b1_db = nc.dram_tensor("b1", [1, HIDDEN], bf16, kind="ExternalInput").ap()
    beta_d = nc.dram_tensor("beta", [1, HIDDEN], f32, kind="ExternalInput").ap()
    sgn_d = nc.dram_tensor("sgn", [1, HIDDEN], f32, kind="ExternalInput").ap()
    bfc_d = nc.dram_tensor("bfc", [1, NUM_CLASSES], f32, kind="ExternalInput").ap()
    idm_d = nc.dram_tensor("idm", [P, P], bf16, kind="ExternalInput").ap()
    iot_d = nc.dram_tensor("iot", [P, P], bf16, kind="ExternalInput").ap()
    out_d = nc.dram_tensor("out", [TPC * P, NUM_CLASSES], f32,
                           kind="ExternalOutput").ap()

    def bcast(src_ap, parts=P):
        return bass.AP(tensor=src_ap.tensor, offset=src_ap.offset,
                       ap=[[0, parts]] + list(src_ap.ap[1:]))

    AL = mybir.AluOpType
    AF = mybir.ActivationFunctionType

    with tile.TileContext(nc) as tc, ExitStack() as ctx:
        consts = ctx.enter_context(tc.tile_pool(name="consts", bufs=1))
        gpool = ctx.enter_context(tc.tile_pool(name="gather", bufs=3))
        ohpool = ctx.enter_context(tc.tile_pool(name="onehot", bufs=2))
        ohupool = ctx.enter_context(tc.tile_pool(name="ohu", bufs=1))
        sp = ctx.enter_context(tc.tile_pool(name="work", bufs=4))
        t1pool = ctx.enter_context(tc.tile_pool(name="t1p", bufs=FCB + 3))
        statp = ctx.enter_context(tc.tile_pool(name="stats", bufs=3))
        pp_ps = ctx.enter_context(tc.tile_pool(name="pp_ps", bufs=2, space="PSUM"))
        agg_ps = ctx.enter_context(tc.tile_pool(name="agg_ps", bufs=2, space="PSUM"))
        tr_ps = ctx.enter_context(tc.tile_pool(name="tr_ps", bufs=2, space="PSUM"))
        fc_ps = ctx.enter_context(tc.tile_pool(name="fc_ps", bufs=2, space="PSUM"))

        W1_s = consts.tile([IN_CH, HIDDEN], bf16)
        nc.sync.dma_start(W1_s[:], w1_d[:])
        Wfc_s = consts.tile([HIDDEN, NUM_CLASSES], bf16)
        nc.sync.dma_start(Wfc_s[:], wfc_d[:])
        B1 = consts.tile([P, HIDDEN], bf16)
        nc.sync.dma_start(B1[:], bcast(b1_db))
        if not beta_zero:
            BETA = consts.tile([P, HIDDEN], f32)
            nc.sync.dma_start(BETA[:], bcast(beta_d))
        if sgn_needed:
            SGN = consts.tile([P, HIDDEN], f32)
            nc.sync.dma_start(SGN[:], bcast(sgn_d))
        if not bfc_zero:
            BFC = consts.tile([P, FCB * NUM_CLASSES], f32)
            nc.sync.dma_start(
                BFC[:], bass.AP(tensor=bfc_d.tensor, offset=bfc_d.offset,
                                ap=[[0, P], [0, FCB], [1, NUM_CLASSES]]))
        ident = consts.tile([P, P], bf16)
        nc.sync.dma_start(ident[:], idm_d[:])
        eps_t = consts.tile([P, FCB], f32)
        nc.vector.memset(eps_t[:], LN_EPS)
        scl_s = consts.tile([P, TPC], f32)
        nc.sync.dma_start(scl_s[:], scl_d[:])

        idx_s = consts.tile([P, NCOLS * 8], i16)
        for gi2, (s2, gb2) in enumerate(grps):
            a = gb2 * CH * 8
            bcol = (gb2 + s2) * CH * 8
            nc.sync.dma_start(idx_s[:, a:bcol], idx_d[:, a:bcol])
        built_base = {}
        bacc_cols = 0
        for gi2, (s2, gb2) in enumerate(grps):
            if not _stream_group(gi2):
                built_base[gi2] = bacc_cols
                bacc_cols += s2 * CH
        BCOLS = max(bacc_cols, 1)
        if any_dve:
            IOT = consts.tile([P, P], bf16)
            nc.sync.dma_start(IOT[:], iot_d[:])
            slot_s = consts.tile([P, NCOLS], bf16)
            nc.sync.dma_start(slot_s[:], slot_d[:])
            nrm_s = consts.tile([P, NCOLS], bf16)
            nc.sync.dma_start(nrm_s[:], nrm_d[:])
            srep = consts.tile([P, BCOLS * REP], bf16)
            nrep = consts.tile([P, BCOLS * REP], bf16)

        out_acc = consts.tile([P, TPC * NUM_CLASSES], f32)

        ohq_v = ohq_d.rearrange("p (t c) -> p t c", c=P)

        t_global = 0
        qn = 0
        fc = None
        nb = 0
        for gi, (s, gbase_tile) in enumerate(grps):
            gbase = gbase_tile * CH       # chunk-column base of this group
            nco = s * CH
            Gg = gpool.tile([P, GS * CH, IN_CH], bf16, tag="Gg")
            gq = GATHER_QUEUES if any_stream else tuple(range(NQ))
            for b in range(NBANK):
                n = s * K[b] * P
                ccol = gbase + s * Koff[b]
                nc.gpsimd.dma_gather(
                    out_ap=Gg[:, s * Koff[b]:s * Koff[b] + s * K[b], :],
                    in_ap=xb[b][:],
                    idxs_ap=idx_s[:, ccol * 8:ccol * 8 + n // 16],
                    num_idxs=n, num_idxs_reg=n, elem_size=IN_CH,
                    single_packet=SP_GATHER, queue_num=gq[qn % len(gq)],
                )
                qn += 1
            ohg = ohpool.tile([P, GS * CH, P], bf16, tag="ohg")
            if _stream_group(gi):
                # uint8 one-hot streamed via HWDGE, cast u8->bf16 on DVE
                sb = stream_base[gi]
                ohu = ohupool.tile([P, GS * CH, P], u8, tag="ohu")
                nc.sync.dma_start(ohu[:, :nco, :], ohq_v[:, sb:sb + nco, :])
                nc.vector.tensor_copy(ohg[:, :nco, :], ohu[:, :nco, :])
            else:
                # built on device: oh[p,c,j] = (iota[j]==slot[p,c]) * q[p,c];
                # slot/q are pre-expanded x8 so every operand has a unit
                # innermost AP step (DVE 2x perf mode).
                bb = built_base[gi]
                sv = srep[:, bb * REP:(bb + nco) * REP]
                nv = nrep[:, bb * REP:(bb + nco) * REP]
                nc.vector.tensor_copy(
                    sv.rearrange("p (c r) -> p c r", r=REP),
                    slot_s[:, gbase:gbase + nco].unsqueeze(2).broadcast_to(
                        [P, nco, REP]))
                nc.vector.tensor_copy(
                    nv.rearrange("p (c r) -> p c r", r=REP),
                    nrm_s[:, gbase:gbase + nco].unsqueeze(2).broadcast_to(
                        [P, nco, REP]))
                ov = ohg[:, :nco, :].rearrange("p c (jb j8) -> p c jb j8",
                                               j8=REP)
                iota_v = bass.AP(tensor=IOT.tensor, offset=IOT[:].offset,
                                 ap=[IOT[:].ap[0], [0, nco],
                                     [REP, P // REP], [1, REP]])
                slot_v = bass.AP(tensor=srep.tensor, offset=sv.offset,
                                 ap=[sv.ap[0], [REP, nco],
                                     [0, P // REP], [1, REP]])
                nrm_v = bass.AP(tensor=nrep.tensor, offset=nv.offset,
                                ap=[nv.ap[0], [REP, nco],
                                    [0, P // REP], [1, REP]])
                nc.vector.tensor_tensor(out=ov, in0=iota_v, in1=slot_v,
                                        op=AL.is_equal)
                nc.vector.tensor_tensor(out=ov, in0=ov, in1=nrm_v,
                                        op=AL.mult)
            for j in range(s):
                t = t_global
                t_global += 1
                Pp = pp_ps.tile([IN_CH, P], f32, space="PSUM")
                mm = 0
                for b in range(NBANK):
                    for kk in range(K[b]):
                        cig = s * Koff[b] + j * K[b] + kk
                        nc.tensor.matmul(Pp[:], lhsT=Gg[:, cig, :],
                                         rhs=ohg[:, j * CH + mm, :],
                                         start=(mm == 0), stop=(mm == CH - 1))
                        mm += 1
                Ps = sp.tile([IN_CH, P], bf16, tag="Ps")
                nc.scalar.activation(out=Ps[:], in_=Pp[:], func=AF.Copy)
                agg = agg_ps.tile([P, HIDDEN], f32, space="PSUM")
                nc.tensor.matmul(agg[:], lhsT=Ps[:], rhs=W1_s[:],
                                 start=True, stop=True)
                jb = t % FCB
                if jb == 0:
                    nb = min(FCB, TPC - t)
                    fc = fc_ps.tile([P, FCB * NUM_CLASSES], f32, space="PSUM")
                    musum_b = statp.tile([P, FCB], f32, tag="musum")
                    s2sum_b = statp.tile([P, FCB], f32, tag="s2sum")
                    rstd_b = statp.tile([P, FCB], f32, tag="rstd")
                    nmr_b = statp.tile([P, FCB], f32, tag="nmr")
                    t1s = []
                # LayerNorm with var = E[x^2] - mu^2; quantization unscale
                # folded into the stt per-partition scalar; per-tile stats
                # batched FCB tiles at a time to amortize tiny-op overhead.
                t1 = t1pool.tile([P, HIDDEN], f32, tag="t1")
                t1s.append(t1)
                nc.vector.scalar_tensor_tensor(
                    out=t1[:], in0=agg[:], scalar=scl_s[:, t:t + 1],
                    in1=B1[:], op0=AL.mult, op1=AL.add,
                    accum_out=musum_b[:, jb:jb + 1])
                sq = sp.tile([P, HIDDEN], f32, tag="sq")
                nc.scalar.activation(out=sq[:], in_=t1[:], func=AF.Square,
                                     accum_out=s2sum_b[:, jb:jb + 1])
                if jb == nb - 1:
                    mm2 = statp.tile([P, FCB], f32, tag="mm2")
                    nc.vector.tensor_tensor(
                        out=mm2[:, :nb], in0=musum_b[:, :nb],
                        in1=musum_b[:, :nb], op=AL.mult)
                    varv = statp.tile([P, FCB], f32, tag="varv")
                    nc.vector.scalar_tensor_tensor(
                        out=varv[:, :nb], in0=mm2[:, :nb],
                        scalar=-1.0 / (HIDDEN * HIDDEN),
                        in1=eps_t[:, :nb], op0=AL.mult, op1=AL.add)
                    nc.vector.scalar_tensor_tensor(
                        out=varv[:, :nb], in0=s2sum_b[:, :nb],
                        scalar=1.0 / HIDDEN,
                        in1=varv[:, :nb], op0=AL.mult, op1=AL.add)
                    nc.scalar.activation(out=rstd_b[:, :nb], in_=varv[:, :nb],
                                         func=AF.Sqrt)
                    nc.vector.reciprocal(out=rstd_b[:, :nb],
                                         in_=rstd_b[:, :nb])
                    nc.vector.scalar_tensor_tensor(
                        out=nmr_b[:, :nb], in0=musum_b[:, :nb],
                        scalar=-1.0 / HIDDEN,
                        in1=rstd_b[:, :nb], op0=AL.mult, op1=AL.mult)
                    for j2 in range(nb):
                        t2 = t - jb + j2
                        t1j = t1s[j2]
                        hr = sp.tile([P, HIDDEN], bf16, tag="hr")
                        if beta_zero and not sgn_needed:
                            nc.scalar.activation(
                                out=hr[:], in_=t1j[:], func=AF.Relu,
                                bias=nmr_b[:, j2:j2 + 1],
                                scale=rstd_b[:, j2:j2 + 1])
                        else:
                            z = sp.tile([P, HIDDEN], f32, tag="z")
                            nc.scalar.activation(
                                out=z[:], in_=t1j[:], func=AF.Identity,
                                bias=nmr_b[:, j2:j2 + 1],
                                scale=rstd_b[:, j2:j2 + 1])
                            if sgn_needed:
                                nc.vector.tensor_tensor(
                                    out=z[:], in0=z[:], in1=SGN[:], op=AL.mult)
                            if not beta_zero:
                                nc.vector.tensor_tensor(
                                    out=z[:], in0=z[:], in1=BETA[:], op=AL.add)
                            nc.scalar.activation(out=hr[:], in_=z[:],
                                                 func=AF.Relu)
                        hrT_ps = tr_ps.tile([HIDDEN, P], bf16, space="PSUM")
                        nc.tensor.transpose(out=hrT_ps[:], in_=hr[:],
                                            identity=ident[:])
                        hrT = sp.tile([HIDDEN, P], bf16, tag="hrT")
                        nc.scalar.activation(out=hrT[:], in_=hrT_ps[:],
                                             func=AF.Copy)
                        nc.tensor.matmul(
                            fc[:, j2 * NUM_CLASSES:(j2 + 1) * NUM_CLASSES],
                            lhsT=hrT[:], rhs=Wfc_s[:], start=True, stop=True)
                    t0 = t - jb
                    dstv = out_acc[:, t0 * NUM_CLASSES:(t + 1) * NUM_CLASSES]
                    srcv = fc[:, :nb * NUM_CLASSES]
                    if bfc_zero:
                        nc.vector.tensor_copy(dstv, srcv)
                    else:
                        nc.vector.tensor_tensor(
                            out=dstv, in0=srcv,
                            in1=BFC[:, :nb * NUM_CLASSES], op=AL.add)

        out_view = out_d.rearrange("(t p) c -> p t c", p=P)
        acc_view = out_acc[:].rearrange("p (t c) -> p t c", c=NUM_CLASSES)
        nc.sync.dma_start(out_view, acc_view)

    nc.compile()
    return nc


def _ensure_ntff_hook():
    import sys, types
    try:
        from antenv.axon_hooks import get_axon_ntff_profile_hook  # noqa: F401
        return
    except ImportError:
        pass
    mod = types.ModuleType("antenv.axon_hooks")
    _hook = [None]
    mod.set_axon_ntff_profile_hook = lambda h: _hook.__setitem__(0, h)
    mod.get_axon_ntff_profile_hook = lambda: _hook[0]
    sys.modules["antenv.axon_hooks"] = mod
    try:
        import antenv
        antenv.axon_hooks = mod
    except ImportError:
        pass
    try:
        from trn_agent_boot.trn_boot import _ntff_profile_via_ctypes
        mod.set_axon_ntff_profile_hook(
            _ntff_profile_via_ctypes("/opt/axon/libaxon_pjrt.so"))
    except Exception:
        pass


# ----------------------------------------------------------------------------
# entry point
# ----------------------------------------------------------------------------
def kernel(x, edge_index, edge_weight, W1, b1, ln_g, ln_b, Wfc, bfc):
    global LAST_RESULTS
    import ml_dtypes
    from concourse.bass_utils import run_bass_kernel_spmd

    bf = ml_dtypes.bfloat16
    x = np.asarray(x, dtype=np.float32)
    meta = _preprocess(edge_index, edge_weight)
    TPC, CH, K = meta["TPC"], meta["CHUNKS"], meta["K"]

    # fold ln_g into Wfc: relu(z*g + b) @ Wfc = relu(sgn*z + b/|g|) @ (|g| Wfc)
    ln_g = np.asarray(ln_g, np.float32)
    ln_b = np.asarray(ln_b, np.float32)
    Wfc = np.asarray(Wfc, np.float32)
    bfc = np.asarray(bfc, np.float32)
    gz = ln_g == 0.0
    absg = np.where(gz, 1.0, np.abs(ln_g))
    beta = np.where(gz, 0.0, ln_b / absg)
    sgn = np.where(gz, 0.0, np.sign(ln_g)).astype(np.float32)
    Wfc_f = Wfc * np.where(gz, 0.0, np.abs(ln_g))[:, None]
    bfc_f = bfc + np.maximum(ln_b, 0.0)[gz] @ Wfc[gz] if gz.any() else bfc
    beta_zero = bool((beta == 0.0).all())
    sgn_needed = bool((sgn != 1.0).any())
    bfc_zero = bool((bfc_f == 0.0).all())

    key = (TPC, CH, K, beta_zero, bfc_zero, sgn_needed,
           SP_GATHER, N_STREAM)
    if key not in _PROGRAM_CACHE:
        _PROGRAM_CACHE[key] = _build_program(TPC, CH, K, beta_zero, bfc_zero,
                                             sgn_needed)
    nc = _PROGRAM_CACHE[key]

    NCOLS = TPC * CH
    grps = _groups(TPC)
    banks = {}
    for b in range(NBANK):
        blk = np.zeros((BANK, IN_CH), dtype=bf)
        seg = x[b * BANK:(b + 1) * BANK]
        blk[:len(seg)] = seg.astype(bf)
        banks[f"xb{b}"] = blk
    common = dict(
        banks,
        W1=np.ascontiguousarray(np.asarray(W1, np.float32).astype(bf)),
        Wfc=np.ascontiguousarray(Wfc_f.astype(bf)),
        b1=np.asarray(b1, np.float32).reshape(1, HIDDEN).astype(bf),
        beta=beta.reshape(1, HIDDEN).astype(np.float32),
        sgn=sgn.reshape(1, HIDDEN),
        bfc=bfc_f.reshape(1, NUM_CLASSES).astype(np.float32),
        idm=np.eye(P, dtype=np.float32).astype(bf),
        iot=np.tile(np.arange(P, dtype=np.float32).astype(bf), (P, 1)),
    )
    # per-core one-hot metadata (quantized values q, exact in bf16)
    oh_lane = meta["oh_lane"]
    oh_mcol = meta["oh_mcol"]
    oh_slot = meta["oh_slot"]
    oh_val = meta["oh_val"]
    core_of = oh_mcol // NCOLS

    # map tile-major metadata col -> streamed DRAM col (or -1)
    col2stream = np.full(NCOLS, -1, dtype=np.int64)
    acc = 0
    for gi, (s, gb) in enumerate(grps):
        if _stream_group(gi):
            col2stream[gb * CH:(gb + s) * CH] = np.arange(acc, acc + s * CH)
            acc += s * CH
    SCOLS = max(acc, 1)

    in_maps = []
    for core in range(N_CORES):
        sl8 = slice(core * NCOLS * 8, (core + 1) * NCOLS * 8)
        m = core_of == core
        lc = oh_mcol[m] - core * NCOLS
        slotm = np.full((P, NCOLS), -1.0, dtype=bf)
        nrmm = np.zeros((P, NCOLS), dtype=bf)
        slotm[oh_lane[m], lc] = oh_slot[m].astype(bf)
        nrmm[oh_lane[m], lc] = oh_val[m].astype(bf)
        ohq = np.zeros((P, SCOLS * P), dtype=np.uint8)
        sc = col2stream[lc]
        ms = sc >= 0
        ohq[oh_lane[m][ms], sc[ms] * P + oh_slot[m][ms]] = \
            oh_val[m][ms].astype(np.uint8)
        scl = np.ascontiguousarray(
            meta["scl"][:, core * TPC:(core + 1) * TPC])
        in_maps.append(dict(
            common,
            idx=np.ascontiguousarray(meta["idx_all"][:, sl8]),
            slotm=slotm,
            nrmm=nrmm,
            ohq=ohq,
            scl=scl,
        ))

    trace = bool(os.environ.get("KERNEL_TRACE"))
    if trace:
        _ensure_ntff_hook()
    res = run_bass_kernel_spmd(nc, in_maps, list(range(N_CORES)), trace=trace)
    LAST_RESULTS = res

    all_rows = np.concatenate([res.results[c]["out"] for c in range(N_CORES)],
                              axis=0)
    rows = meta["node_tile"] * P + meta["node_slot"]
    return np.ascontiguousarray(all_rows[rows])
